# revision 46
# baseline (speedup 1.0000x reference)
"""CondConv2d (MoE-routed conv) Trainium2 kernel.

Reference computation (per sample b):
    pooled  = mean(x[b], HW)                          [C]
    r       = sigmoid(pooled @ Wr^T + br)             [E]
    w_b     = (r @ weight).reshape(O, C, 3, 3)
    bias_b  = r @ bias                                [O]
    out[b]  = conv2d(x[b], w_b, pad=1) + bias_b

Sharding: data-parallel over batch, 4 samples per core on 8 cores; the
small expert weight bank is replicated to every core (no collectives).

Per-core dataflow (one Tile program):
  - x arrives width-padded (56 -> 58 with zero cols) in bf16; the conv is
    9 shifted accumulating PE matmuls per (o-chunk, row-block), K = C = 128,
    N = 8 rows * 56 cols = 448, PSUM fp32.
  - routing runs entirely off the PE so weight-gen for sample b+1 can
    pipeline under sample b's conv matmuls: DVE free-dim reduce (sum over
    H*W), DVE per-partition scalar mul, GPSIMD partition_all_reduce for
    the logits, ACT sigmoid; per-sample bias via elementwise mul +
    free-dim reduce against a transposed bias bank.
  - per-sample conv weights, per (o-chunk, tap-range chunk): experts 0-5
    on DVE (tensor_scalar mul + 5 fused scalar_tensor_tensor FMAs;
    pointer-scalar ops are DVE-only on HW), experts 6-7 on GPSIMD as
    tensor_tensor muls with a stride-0 broadcast of the routing scalar,
    one DVE add combines the partials and writes the bf16 lhsT directly.
  - per-sample bias is fused into the ACT drain of conv PSUM banks.
  - the expert bank is loaded bf16 as one combined 8-expert DMA per
    (o-chunk, tap-range) so weight-gen inputs arrive early; sample 0's
    first chunk is a single tap so the first conv matmul starts ~11us in.

Conv matmul order: the first (sample, o-chunk) group is tap-major (so the
staged weight chunks let the PE start early); all other groups are
BLOCK-major — each row block's 9 taps run back-to-back and its PSUM bank
drains immediately, spreading bank releases evenly (tap-major bunched all
7 drains at the group boundary, stalling the next group's first matmul on
the ACT counter semaphore and re-throttling the PE pstate every group).

Cost-model timeline (instruction_cost_v2): ~114us/core vs a ~94us bf16
PE roofline (504 matmuls of N=448) and ~59us of DMA; remaining losses are
~11us startup (sample-0 routing chain), ~2us first-group weight handoff,
~4us tail drain + end barrier.
"""

import contextlib
import sys

sys.path.insert(0, "/opt/trn_rl_repo")

import numpy as np
import ml_dtypes

import concourse.bass as bass  # noqa: F401
import concourse.bass_isa as bass_isa
import concourse.mybir as mybir
from concourse.tile import TileContext
from concourse.tile_rust import add_dep_helper
from concourse import bass_utils, bacc

F32 = mybir.dt.float32
BF16 = mybir.dt.bfloat16

B, C, H, W = 32, 128, 56, 56
OUT_C, KH, KW = 256, 3, 3
E = 8
N_CORES = 8
BPC = B // N_CORES          # samples per core
WP = W + 2                  # width padded with one zero col each side
HWP = H * WP                # 3248 padded pixels
NPIX = H * W                # 3136 output pixels
POS = KH * KW               # 9 kernel positions
RB = 8                      # output rows per PSUM block
NBLK = H // RB              # 7 row blocks
NB = RB * W                 # 448 = PSUM tile free size
OCC = OUT_C // 128          # 2 output-channel chunks
PK = POS * 128              # 1152 weight elems per partition per (e, oc)

# position order: full-coverage center tap first (start=True covers the
# whole PSUM region), remaining taps accumulate partial row ranges.
# The weight bank's pos axis is host-reordered to match, so tap t's lhsT
# is wb16[:, t*128:(t+1)*128].
POS_ORDER = [(1, 1), (0, 0), (0, 1), (0, 2), (1, 0), (1, 2), (2, 0), (2, 1), (2, 2)]

# weight-gen (and the bank DMA) is split into two tap-range chunks so the
# first conv matmuls only wait for taps 0-3
GEN_SPLITS = [(0, 4), (4, POS)]

_CACHED_NC = None


def _build_nc(repeat=1):
    """Build the SPMD program. `repeat` re-runs the whole per-sample
    pipeline (for the scale-test timing harness only — outputs are simply
    rewritten)."""
    nc = bacc.Bacc("TRN2", target_bir_lowering=False, debug=False,
                   num_devices=N_CORES)

    x_d = nc.dram_tensor("x", [BPC, C, HWP], BF16, kind="ExternalInput").ap()
    # bank layout: [i, oc, e, pos*128] so each (oc, e) chunk is contiguous
    w_d = nc.dram_tensor("wbank", [C, OCC, E, PK], BF16,
                         kind="ExternalInput").ap()
    rwT_d = nc.dram_tensor("rwT", [C, E], F32, kind="ExternalInput").ap()
    rbb_d = nc.dram_tensor("rbias_bc", [C, E], F32, kind="ExternalInput").ap()
    # biasT[o', oc, e] = bias[e, oc*128 + o']
    biasT_d = nc.dram_tensor("biasT", [128, OCC * E], F32,
                             kind="ExternalInput").ap()
    out_d = nc.dram_tensor("out", [BPC, OUT_C, NPIX], F32,
                           kind="ExternalOutput").ap()

    with TileContext(nc) as tc:
        with (
            tc.tile_pool(name="const", bufs=1) as cpool,
            tc.tile_pool(name="wbank", bufs=OCC) as wpool,
            tc.tile_pool(name="xin", bufs=4) as xpool,
            tc.tile_pool(name="wgen", bufs=3) as gpool,
            tc.tile_pool(name="wbf", bufs=6) as wbfpool,
            tc.tile_pool(name="outp", bufs=4) as opool,
            tc.tile_pool(name="small", bufs=2) as spool,
            tc.tile_pool(name="conv_ps", bufs=NBLK, space="PSUM") as pspool,
            tc.tile_pool(name="warm_ps", bufs=1, space="PSUM") as wpps,
        ):
            # PE warm-up: the first ~11us are otherwise PE-idle (sample 0's
            # conv depends on its routing); dependency-free dummy matmuls
            # into a scratch PSUM bank absorb the pstate/HAM ramp so the
            # real conv starts at full clock.
            warm = cpool.tile([128, 512], BF16, tag="warm")
            nc.vector.memset(warm[:, :], 0.0)
            wps_t = wpps.tile([128, 512], F32, tag="wps", name="warm_psum")
            for _wi in range(40):
                nc.tensor.matmul(wps_t[:, :], lhsT=warm[:, :128],
                                 rhs=warm[:, :], start=True, stop=True)
            # first sample's input before everything else: its two DMA
            # halves pipeline with the two pooled partial reduces while the
            # weight bank streams in behind it.
            x_tiles = {}
            x_tiles[0] = xpool.tile([C, HWP], BF16, name="xt_0", tag="xt")
            HH = HWP // 2
            nc.sync.dma_start(out=x_tiles[0][:, :HH], in_=x_d[0][:, :HH])
            nc.sync.dma_start(out=x_tiles[0][:, HH:], in_=x_d[0][:, HH:])

            # --- replicated constants -------------------------------------
            rwT = cpool.tile([C, E], F32, tag="rwT")
            nc.sync.dma_start(out=rwT[:, :], in_=rwT_d[:, :])
            rbb = cpool.tile([C, E], F32, tag="rbb")
            nc.sync.dma_start(out=rbb[:, :], in_=rbb_d[:, :])
            biasT = cpool.tile([C, OCC * E], F32, tag="biasT")
            nc.sync.dma_start(out=biasT[:, :], in_=biasT_d[:, :])
            biasT_v = biasT[:, :].rearrange("c (o e) -> c o e", e=E)

            # bank chunks keyed (oc, ci): ONE combined DMA per chunk
            # covering all 8 experts, so each chunk's weight-gen inputs
            # arrive together and early (HWDGE descriptor-gen costs
            # ~0.6us per dma_start)
            wbank = {}
            for oc in range(OCC):
                for ci, (t0, t1) in enumerate(GEN_SPLITS):
                    sz = (t1 - t0) * 128
                    wt = wpool.tile([C, E * sz], BF16,
                                    name=f"wt_{oc}_{ci}", tag=f"wt{ci}")
                    wv = wt[:, :].rearrange("c (e s) -> c e s", e=E)
                    nc.sync.dma_start(
                        out=wv[:, :, :],
                        in_=w_d[:, oc, :, t0 * 128:t1 * 128])
                    wbank[(oc, ci)] = wv

            for rep, b in [(r, bb_) for r in range(repeat)
                           for bb_ in range(BPC)]:
                # --- input ------------------------------------------------
                key = (rep, b)
                if rep == 0 and b == 0:
                    x_tiles[key] = x_tiles.pop(0)
                if key not in x_tiles:
                    x_tiles[key] = xpool.tile([C, HWP], BF16,
                                              name=f"xt_{rep}_{b}", tag="xt")
                    HH2 = HWP // 2
                    nc.sync.dma_start(out=x_tiles[key][:, :HH2],
                                      in_=x_d[b][:, :HH2])
                    nc.sync.dma_start(out=x_tiles[key][:, HH2:],
                                      in_=x_d[b][:, HH2:])
                xt = x_tiles[key]
                xv = xt[:, :].rearrange("c (h w) -> c h w", w=WP)

                # --- routing (no PE involvement) -------------------------
                # pooling on ACT via the activation accumulator, in two
                # halves so the first can start after half the x DMA;
                # final [C,2] -> [C,1] reduce is a tiny DVE op
                HH = HWP // 2
                scr = spool.tile([C, HWP // 2], BF16, tag="scr",
                                 name=f"scr_{rep}_{b}")
                ph = spool.tile([C, 2], F32, tag="ph", name=f"ph_{rep}_{b}")
                nc.scalar.activation(scr[:, :], xt[:, :HH],
                                     mybir.ActivationFunctionType.Copy,
                                     accum_out=ph[:, 0:1])
                if rep == 0 and b == 0:
                    # startup: second half on DVE, in parallel with ACT
                    nc.vector.reduce_sum(out=ph[:, 1:2], in_=xt[:, HH:],
                                         axis=mybir.AxisListType.X)
                else:
                    # steady state: keep DVE free for the FMA chains
                    nc.scalar.activation(scr[:, :], xt[:, HH:],
                                         mybir.ActivationFunctionType.Copy,
                                         accum_out=ph[:, 1:2])
                pooled = spool.tile([C, 1], F32, tag="pooled",
                                    name=f"pooled_{rep}_{b}")
                nc.vector.reduce_sum(out=pooled[:, :], in_=ph[:, :],
                                     axis=mybir.AxisListType.X)

                # per-partition partial logits, then all-reduce over
                # partitions so every partition holds the full logits
                rp = spool.tile([C, E], F32, tag="rp", name=f"rp_{rep}_{b}")
                nc.vector.tensor_scalar_mul(out=rp[:, :], in0=rwT[:, :],
                                            scalar1=pooled[:, :])
                nc.gpsimd.partition_all_reduce(rp[:, :], rp[:, :], C,
                                               bass_isa.ReduceOp.add)
                lg = spool.tile([C, E], F32, tag="lg", name=f"lg_{rep}_{b}")
                nc.vector.tensor_add(out=lg[:, :], in0=rp[:, :],
                                     in1=rbb[:, :])
                r_bc = spool.tile([C, E], F32, tag="rbc_s", name=f"rbc_{rep}_{b}")
                nc.scalar.activation(r_bc[:, :], lg[:, :],
                                     mybir.ActivationFunctionType.Sigmoid)

                # per-sample output bias b_b = r @ bias   -> [128, occ]
                bbt = spool.tile([C, OCC * E], F32, tag="bbt",
                                 name=f"bbt_{rep}_{b}")
                bbt_v = bbt[:, :].rearrange("c (o e) -> c o e", e=E)
                for oc in range(OCC):
                    nc.vector.tensor_mul(out=bbt_v[:, oc, :],
                                         in0=biasT_v[:, oc, :],
                                         in1=r_bc[:, :])
                bb = spool.tile([128, OCC], F32, tag="bb", name=f"bb_{rep}_{b}")
                nc.vector.reduce_sum(out=bb[:, :], in_=bbt_v[:, :, :],
                                     axis=mybir.AxisListType.X)

                for oc in range(OCC):
                    # --- per-sample conv weights for this o-chunk ---------
                    # experts 0-5 on DVE (pointer-scalar FMAs are DVE-only
                    # on hardware); experts 6-7 on GPSIMD as plain
                    # tensor_tensor muls with a stride-0 broadcast of the
                    # routing scalar; one DVE add combines the partials and
                    # writes the bf16 lhsT directly.  Each tap-range chunk
                    # gets its own tiles so the conv's first LDWEIGHTS only
                    # depends on chunk 0.
                    # the kernel's overall critical path runs through the
                    # very first weight chunk: split it extra-fine (single
                    # tap) there so the first conv matmul starts ASAP
                    first = rep == 0 and b == 0 and oc == 0
                    splits = ([(0, 1), (1, GEN_SPLITS[0][1])] + GEN_SPLITS[1:]
                              if first else GEN_SPLITS)
                    wb16 = {}
                    prev_comb = prev_sB = None
                    for ci, (t0, t1) in enumerate(splits):
                        hp = (tc.high_priority() if first and ci == 0
                              else contextlib.nullcontext())
                        # source DMA chunk (keyed by GEN_SPLITS) + offset
                        src = 0 if t1 <= GEN_SPLITS[0][1] else 1
                        lo = (t0 - (0 if src == 0 else GEN_SPLITS[0][1])) * 128
                        sz = (t1 - t0) * 128
                        rtag = f"{t0}_{t1}"
                        with hp:
                            wfA = gpool.tile([C, sz], F32, tag=f"wfA{rtag}",
                                             name=f"wfA{ci}_{rep}_{b}_{oc}")
                            mul_i = nc.vector.tensor_scalar_mul(
                                out=wfA[:, :],
                                in0=wbank[(oc, src)][:, 0, lo:lo + sz],
                                scalar1=r_bc[:, 0:1])
                            if first and prev_comb is not None:
                                # keep the startup-critical chunk chains
                                # strictly ordered on DVE
                                add_dep_helper(mul_i.ins, prev_comb.ins,
                                               sync=False,
                                               reason="startup chunk order")
                            for e in range(1, 6):
                                nc.vector.scalar_tensor_tensor(
                                    out=wfA[:, :],
                                    in0=wbank[(oc, src)][:, e, lo:lo + sz],
                                    scalar=r_bc[:, e:e + 1], in1=wfA[:, :],
                                    op0=mybir.AluOpType.mult,
                                    op1=mybir.AluOpType.add)
                            t6 = gpool.tile([C, sz], F32, tag=f"t6{rtag}",
                                            name=f"t6{ci}_{rep}_{b}_{oc}")
                            t6_i = nc.gpsimd.tensor_mul(
                                out=t6[:, :],
                                in0=wbank[(oc, src)][:, 6, lo:lo + sz],
                                in1=r_bc[:, 6:7].broadcast_to([C, sz]))
                            if first and prev_sB is not None:
                                add_dep_helper(t6_i.ins, prev_sB.ins,
                                               sync=False,
                                               reason="startup chunk order")
                            t7 = gpool.tile([C, sz], F32, tag=f"t7{rtag}",
                                            name=f"t7{ci}_{rep}_{b}_{oc}")
                            nc.gpsimd.tensor_mul(
                                out=t7[:, :],
                                in0=wbank[(oc, src)][:, 7, lo:lo + sz],
                                in1=r_bc[:, 7:8].broadcast_to([C, sz]))
                            prev_sB = nc.gpsimd.tensor_add(
                                out=t6[:, :], in0=t6[:, :], in1=t7[:, :])
                            wchunk = wbfpool.tile([C, sz], BF16,
                                                  tag=f"wb16{rtag}",
                                                  name=f"wb16{ci}_{rep}_{b}_{oc}")
                            prev_comb = nc.vector.tensor_add(
                                out=wchunk[:, :], in0=wfA[:, :],
                                in1=t6[:, :])
                            wb16[ci] = wchunk

                    # --- conv: 9 shifted matmuls per row-block ------------
                    def tap_lhsT(idx):
                        for ci_, (t0_, t1_) in enumerate(splits):
                            if t0_ <= idx < t1_:
                                return wb16[ci_][:, (idx - t0_) * 128:
                                                 (idx - t0_ + 1) * 128]

                    def tap_mm(ps, blk, idx, dy, dx):
                        r0 = blk * RB
                        j0 = max(0, 1 - dy - r0)
                        j1 = min(RB, 57 - dy - r0)
                        rs = r0 + j0 + dy - 1
                        rhs = xv[:, rs:rs + (j1 - j0), dx:dx + W]
                        nc.tensor.matmul(ps[:, j0 * W:j1 * W],
                                         lhsT=tap_lhsT(idx), rhs=rhs,
                                         start=(idx == 0),
                                         stop=(idx == POS - 1))

                    def drain(ps, blk):
                        ot = opool.tile([128, NB], F32, tag="ot",
                                        name=f"ot_{rep}_{b}_{oc}_{blk}")
                        nc.scalar.add(out=ot[:, :], in_=ps[:, :],
                                      add=bb[:, oc:oc + 1])
                        nc.sync.dma_start(
                            out=out_d[b, oc * 128:(oc + 1) * 128,
                                      blk * NB:(blk + 1) * NB],
                            in_=ot[:, :])

                    if first:
                        # tap-major: the conv can start on the single-tap
                        # first weight chunk while later chunks generate
                        ps_tiles = [pspool.tile([128, NB], F32, tag="cps",
                                                name=f"cps_{rep}_{b}_{oc}_{blk}")
                                    for blk in range(NBLK)]
                        for idx, (dy, dx) in enumerate(POS_ORDER):
                            for blk in range(NBLK):
                                tap_mm(ps_tiles[blk], blk, idx, dy, dx)
                        for blk in range(NBLK):
                            drain(ps_tiles[blk], blk)
                    else:
                        # block-major: each block's 9 taps run back-to-back
                        # and its PSUM bank drains immediately, so bank
                        # releases spread evenly instead of bunching at the
                        # group boundary (which stalled the next group's
                        # first matmul on the ACT drain backlog)
                        for blk in range(NBLK):
                            ps = pspool.tile([128, NB], F32, tag="cps",
                                             name=f"cps_{rep}_{b}_{oc}_{blk}")
                            for idx, (dy, dx) in enumerate(POS_ORDER):
                                tap_mm(ps, blk, idx, dy, dx)
                            drain(ps, blk)

    nc.compile()
    return nc


def _get_nc():
    global _CACHED_NC
    if _CACHED_NC is None:
        _CACHED_NC = _build_nc()
    return _CACHED_NC


def _prepare_in_maps(x, weight, routing_weight, routing_bias, bias):
    xp = np.zeros((B, C, H, WP), dtype=np.float32)
    xp[:, :, :, 1:1 + W] = x
    xp = xp.astype(ml_dtypes.bfloat16).reshape(B, C, HWP)

    # weight flat order is (o, i, kh, kw) with o = oc*128 + o'.
    # rearrange to [i, oc, e, pos, o'] so each (oc, e) chunk is contiguous,
    # with the pos axis permuted into conv tap order (POS_ORDER).
    tap_pos = [dy * 3 + dx for dy, dx in POS_ORDER]
    wr = weight.reshape(E, OCC, 128, C, POS).transpose(3, 1, 0, 4, 2)
    wr = wr[:, :, :, tap_pos, :]
    wr = np.ascontiguousarray(wr).astype(ml_dtypes.bfloat16)
    wr = wr.reshape(C, OCC, E, PK)

    rwT = np.ascontiguousarray(routing_weight.T / NPIX, dtype=np.float32)
    rbb = np.broadcast_to(routing_bias.reshape(1, E), (C, E))
    rbb = np.ascontiguousarray(rbb, dtype=np.float32)
    # biasT[o', oc, e] = bias[e, oc*128 + o']
    biasT = bias.T.reshape(OCC, 128, E).transpose(1, 0, 2)
    biasT = np.ascontiguousarray(biasT, dtype=np.float32).reshape(128, OCC * E)

    in_maps = []
    for c in range(N_CORES):
        in_maps.append({
            "x": np.ascontiguousarray(xp[c * BPC:(c + 1) * BPC]),
            "wbank": wr,
            "rwT": rwT,
            "rbias_bc": rbb,
            "biasT": biasT,
        })
    return in_maps


def kernel(x, weight, routing_weight, routing_bias, bias, _trace=False):
    nc = _get_nc()
    in_maps = _prepare_in_maps(np.asarray(x, dtype=np.float32),
                               np.asarray(weight, dtype=np.float32),
                               np.asarray(routing_weight, dtype=np.float32),
                               np.asarray(routing_bias, dtype=np.float32),
                               np.asarray(bias, dtype=np.float32))
    res = bass_utils.run_bass_kernel_spmd(
        nc, in_maps, core_ids=list(range(N_CORES)), trace=_trace)
    out = np.concatenate([res.results[c]["out"] for c in range(N_CORES)], axis=0)
    out = out.reshape(B, OUT_C, H, W)
    if _trace:
        kernel.last_results = res
    return out
